# revision 13
# baseline (speedup 1.0000x reference)
"""ColumnParallelLinearWithDelta: GPTQ-int4 LoRA-delta matmul on 8 trn2 cores.

out[d] = x @ dequant(qweight[d], qzeros[d], scales[d]) + x @ base_weight.T

Sharding: column-parallel — out_features (4096) split into 8 slices of 512,
one per NeuronCore; x replicated. Each core computes its [8, 256, 512] slice
of the delta stack plus the shared base output; the host adds base and the
K=32 zeros/centering correction during the unshard (exact f32, O(output)
plus a tiny [T,G]x[G,OUT] correction matmul per adapter).

Mixed-precision contraction (per core, out-col slice ns):
  W[k, n] = s[g(k), n] * (w4[k, n] - (z4[g(k), n] + 1)),  g(k) = k // 128

  k in [0, 3072): fp16 path:
    one tensor_scalar (>> 4sh & 0xF) per nibble plane over [128, 3072],
    tensor_tensor * (64*s) -> fp16 planes feed N=512 matmuls (rhs
    moving) against stationary fp16 x tiles; 48 matmuls/adapter.

  k in [3072, 4096): fp8 DoubleRow path:
    the host pre-dequantizes e4m3 planes p8 = e4m3(64*s*(w4-8)) — same
    HBM bytes as packed-int4 + expanded scales (1B/elem) but zero DVE
    work — and e4m3 x tiles paired two k-rows per PE cell
    (perf_mode=DoubleRow, lhsT [128,2,128], rhs [128,2,512]): each
    matmul contracts 256 k-rows in the cycles of 128, halving PE time
    for this range; 8 matmuls/adapter. The -8 nibble centering halves
    the e4m3 rounding error; its linear term and the GPTQ zeros fold
    into the host-side correction.

  The base matmul runs fully in DoubleRow e4m3 (wb8 = e4m3(64*bw), x
  pairs): 32 matmuls instead of 64. Everything accumulates at 64x scale
  in f32 PSUM; ScalarE drains apply 1/64 and write fp16 outputs.

Schedule: short N=128 PE warm-up (HAM un-throttle) -> adapter 0 (DR
matmuls first: no DVE dependency) -> base stream (pure-PE window that
lets DVE run an adapter ahead) -> adapters 1..7 with next-adapter weight
DMAs issued BEFORE the current adapter's matmul tail (keeps the sync
HWDGE ring's FIFO from serializing weights behind output completions).
Output DMAs ride the scalar ring; the final drain alternates rings and
adapter 7 runs t-half-major so th0's drain+DMA overlap th1's matmuls.
"""

import numpy as np
import ml_dtypes

# ---- problem constants (hardcoded; kernel.py must be self-contained) ----
T = 256          # tokens
IN = 4096        # in_features
OUT = 4096       # out_features
D = 8            # adapters
GROUP = 128      # quant group size
G = IN // GROUP  # 32 groups
NCORES = 8
NC_OUT = OUT // NCORES   # 512 out cols per core
NCH = 8                  # contraction chunks of 64 packed rows (x2 e-halves)
C16 = 6                  # chunks 0..5: fp16 path
FD16 = C16 * NC_OUT      # 3072 halfwords per partition (fp16 class)
NSLOT = 4                # adapter DR slots: (c in {6,7}) x (sh'' in {0,1})
NBSLOT = 16              # base DR slots: all (c, sh'')
SCL = 64.0               # psum scale (e4m3 range centering)

_PROGRAM_CACHE: dict = {}


def _build_program():
    import concourse.bacc as bacc
    import concourse.mybir as mybir
    import concourse.tile as tile

    nc = bacc.Bacc("TRN2", target_bir_lowering=False, debug=False)

    fp16 = mybir.dt.float16
    fp8 = mybir.dt.float8e4
    d_xt = nc.dram_tensor("xt", (128, C16 * 4 * T), fp16,
                          kind="ExternalInput")
    d_xt8 = nc.dram_tensor("xt8", (128, NBSLOT * 2 * 256), fp8,
                           kind="ExternalInput")
    d_qw16 = nc.dram_tensor(
        "qw16", (D, 2, 128, FD16 // 2), mybir.dt.int16, kind="ExternalInput"
    )
    d_s2 = nc.dram_tensor("s2", (D, 2, 128, FD16 // 2), fp16,
                          kind="ExternalInput")
    d_p8 = nc.dram_tensor("p8", (D, 128, NSLOT * 1024), fp8,
                          kind="ExternalInput")
    d_wb8 = nc.dram_tensor("wb8", (2, 128, 8 * 1024), fp8,
                           kind="ExternalInput")
    d_out = nc.dram_tensor("out", (D, T, NC_OUT), fp16, kind="ExternalOutput")
    d_outb = nc.dram_tensor("outb", (T, NC_OUT), fp16, kind="ExternalOutput")

    AT = mybir.AluOpType
    AF = mybir.ActivationFunctionType
    DR = mybir.MatmulPerfMode.DoubleRow

    with tile.TileContext(nc) as tc:
        with (
            tc.tile_pool(name="const", bufs=1) as cpool,
            tc.tile_pool(name="qw", bufs=2) as qpool,
            tc.tile_pool(name="s2", bufs=2) as spool,
            tc.tile_pool(name="p8", bufs=2) as p8pool,
            tc.tile_pool(name="vr", bufs=1) as vrpool,
            tc.tile_pool(name="v", bufs=2) as vpool,
            tc.tile_pool(name="wb", bufs=2) as wpool,
            tc.tile_pool(name="outp", bufs=4) as opool,
            tc.tile_pool(name="ps", bufs=2, space="PSUM") as ppool,
            tc.tile_pool(name="psb", bufs=1, space="PSUM") as pbpool,
        ):
            xt_sb = cpool.tile([128, C16 * 4 * T], fp16)
            xt8_sb = cpool.tile([128, NBSLOT * 2 * 256], fp8)
            warm_sb = cpool.tile([128, 640], fp16)

            def xt_tile(c, sh, th):
                off = (c * 4 + sh) * T + th * 128
                return xt_sb[:, off:off + 128]

            def xt8_pair(slot, th):
                off = (slot * 2 + th) * 256
                return xt8_sb[:, off:off + 256].rearrange(
                    "p (two m) -> p two m", two=2)

            def load_adapter(d):
                """DMA an adapter's weight streams. For adapter 0 the
                dequant pipeline is the latency-critical path (warm-up
                matmuls hide the DR planes), so qw/s2 go first; later
                adapters put the e4m3 planes first — they gate the
                adapter's first matmuls."""
                p8_t = p8pool.tile([128, NSLOT * 1024], fp8, name="p8_t")
                qw_t = qpool.tile([128, FD16], mybir.dt.int16, name="qw_t")
                s2_t = spool.tile([128, FD16], fp16, name="s2_t")
                hs = FD16 // 2
                if d > 0:
                    nc.sync.dma_start(p8_t[:], d_p8[d, :, :])
                for h in range(2):
                    nc.sync.dma_start(qw_t[:, h * hs:(h + 1) * hs],
                                      d_qw16[d, h, :, :])
                    nc.sync.dma_start(s2_t[:, h * hs:(h + 1) * hs],
                                      d_s2[d, h, :, :])
                    if d == 0 and h == 0:
                        nc.sync.dma_start(p8_t[:], d_p8[d, :, :])
                return p8_t, qw_t, s2_t

            def dr_mms(d, ps, p8_t, start):
                """8 DoubleRow matmuls (k in [3072, 4096)): no DVE dep."""
                for slot in range(NSLOT):
                    rhs = p8_t[:, slot * 1024:(slot + 1) * 1024].rearrange(
                        "p (two n) -> p two n", two=2)
                    for th in range(2):
                        nc.tensor.matmul(
                            ps[th][:], lhsT=xt8_pair(12 + slot, th), rhs=rhs,
                            start=(start and slot == 0), stop=False,
                            perf_mode=DR,
                        )

            def dequant16(d, qw_t, s2_t, n_parts=1):
                """fp16-class dequant: 4 TS + 4 TT over [128, 3072]."""
                vs = {}
                cs = FD16 // n_parts
                for part in range(n_parts):
                    c0 = part * cs
                    for sh in range(4):
                        vr = vrpool.tile([128, FD16], mybir.dt.int16,
                                         tag=f"vr{sh}", name=f"vr{sh}")
                        ts_i = nc.vector.tensor_scalar(
                            out=vr[:, c0:c0 + cs], in0=qw_t[:, c0:c0 + cs],
                            scalar1=4 * sh, scalar2=0xF,
                            op0=AT.logical_shift_right, op1=AT.bitwise_and,
                        )
                        v = vpool.tile([128, FD16], fp16, tag=f"v{sh}",
                                       name=f"v{sh}")
                        nc.vector.tensor_tensor(
                            out=v[:, c0:c0 + cs], in0=vr[:, c0:c0 + cs],
                            in1=s2_t[:, c0:c0 + cs], op=AT.mult
                        )
                        vs[(part, sh)] = v
                        if d == 0 and part == 0 and sh == 0:
                            first_ops[0] = ts_i
                return vs

            def fp16_mms(ps, vs, n_parts, ths, stop=False):
                ch_per = C16 // n_parts
                for part in range(n_parts):
                    for sh in range(4):
                        v = vs[(part, sh)]
                        for c in range(part * ch_per, (part + 1) * ch_per):
                            rhs = v[:, c * NC_OUT:(c + 1) * NC_OUT]
                            last = (part == n_parts - 1 and sh == 3
                                    and c == (part + 1) * ch_per - 1)
                            for th in ths:
                                nc.tensor.matmul(
                                    ps[th][:], lhsT=xt_tile(c, sh, th),
                                    rhs=rhs, start=False,
                                    stop=(stop and last),
                                )

            def drain_th(d, ps, th, ring_sync=False):
                """1/64-scaled ScalarE drain + fp16 output DMA."""
                o_t = opool.tile([128, NC_OUT], fp16, name="o_t")
                nc.scalar.activation(o_t[:], ps[th][:], AF.Identity,
                                     scale=1.0 / SCL)
                eng = nc.sync if ring_sync else nc.scalar
                eng.dma_start(d_out[d, th * 128:(th + 1) * 128, :], o_t[:])

            # ---- schedule ----
            first_ops = [None]
            loaded = load_adapter(0)
            nc.scalar.dma_start(xt8_sb[:], d_xt8[:])
            for q in range(4):
                nc.scalar.dma_start(
                    xt_sb[:, q * 6 * T:(q + 1) * 6 * T],
                    d_xt[:, q * 6 * T:(q + 1) * 6 * T])

            # PE warm-up: short matmuls flip the HAM clock gate to 2.4GHz
            # while the first weight transfers land
            nc.gpsimd.memset(warm_sb[:], 0.0)
            with tc.tile_pool(name="warmps", bufs=1, space="PSUM") as wpsp:
                warm_ps = wpsp.tile([128, 128], mybir.dt.float32)
                for _ in range(52):
                    nc.tensor.matmul(
                        warm_ps[:], lhsT=warm_sb[:, :128],
                        rhs=warm_sb[:, 128:256],
                        start=True, stop=True,
                    )

            ps0 = [ppool.tile([128, NC_OUT], mybir.dt.float32, tag=f"ps{t}",
                              name=f"ps{t}") for t in range(2)]
            p8_0, qw_0, s2_0 = loaded
            dr_mms(0, ps0, p8_0, start=True)
            vs0 = dequant16(0, qw_0, s2_0, n_parts=2)
            # base weights stream behind adapter 0's (sync-ring FIFO
            # already orders them after adapter 0's weights)
            wb_t = [wpool.tile([128, 8 * 1024], fp8, name=f"wb_t{h}",
                               tag=f"wb{h}") for h in range(2)]
            for h in range(2):
                nc.sync.dma_start(wb_t[h][:], d_wb8[h, :, :])
            loaded = load_adapter(1)
            fp16_mms(ps0, vs0, 2, (0, 1), stop=True)
            drain_th(0, ps0, 0)
            drain_th(0, ps0, 1)

            # base DoubleRow stream: DMA-only (no dequant), fills the PE
            # while DVE dequants adapter 1
            ps_b = [pbpool.tile([128, NC_OUT], mybir.dt.float32, tag=f"psb{t}",
                                name=f"psb{t}") for t in range(2)]
            for slot in range(NBSLOT):
                wt = wb_t[slot // 8]
                so = (slot % 8) * 1024
                rhs = wt[:, so:so + 1024].rearrange(
                    "p (two n) -> p two n", two=2)
                for th in range(2):
                    nc.tensor.matmul(
                        ps_b[th][:], lhsT=xt8_pair(slot, th), rhs=rhs,
                        start=(slot == 0), stop=(slot == NBSLOT - 1),
                        perf_mode=DR,
                    )
            for th in range(2):
                ob_t = opool.tile([128, NC_OUT], fp16, name="ob_t")
                nc.scalar.activation(ob_t[:], ps_b[th][:], AF.Identity,
                                     scale=1.0 / SCL)
                nc.scalar.dma_start(d_outb[th * 128:(th + 1) * 128, :],
                                    ob_t[:])

            for d in range(1, D):
                ps = [ppool.tile([128, NC_OUT], mybir.dt.float32, tag=f"ps{t}",
                                 name=f"ps{t}") for t in range(2)]
                p8_t, qw_t, s2_t = loaded
                dr_mms(d, ps, p8_t, start=True)
                vs = dequant16(d, qw_t, s2_t)
                if d < D - 1:
                    # prefetch next adapter's weights ahead of this
                    # adapter's matmul tail (sync-ring FIFO)
                    loaded = load_adapter(d + 1)
                    fp16_mms(ps, vs, 1, (0, 1), stop=True)
                    drain_th(d, ps, 0)
                    drain_th(d, ps, 1)
                else:
                    # last adapter: th-major so th0's drain + output DMA
                    # overlap th1's matmuls; the final drain is split in
                    # halves pipelined across both HWDGE rings to shrink
                    # the kernel tail
                    fp16_mms(ps, vs, 1, (0,), stop=True)
                    drain_th(d, ps, 0)
                    fp16_mms(ps, vs, 1, (1,), stop=True)
                    for half in range(2):
                        o_t = opool.tile([128, NC_OUT // 2], fp16,
                                         name="o_t")
                        cs = slice(half * 256, (half + 1) * 256)
                        nc.scalar.activation(o_t[:], ps[1][:, cs],
                                             AF.Identity, scale=1.0 / SCL)
                        eng = nc.scalar if half == 0 else nc.sync
                        eng.dma_start(d_out[d, 128:256, cs], o_t[:])

    nc.compile()
    return nc


def _prep_inputs(x, base_weight, qweight, qzeros, scales):
    """Host-side layout prep. Returns (per-core input maps, host corr)."""
    x = np.asarray(x, dtype=np.float32)
    base_weight = np.asarray(base_weight, dtype=np.float32)
    qweight = np.asarray(qweight, dtype=np.int32)
    qzeros = np.asarray(qzeros, dtype=np.int32)
    scales = np.asarray(scales, dtype=np.float32)
    e4m3 = ml_dtypes.float8_e4m3

    # stationary fp16 x tiles (chunks 0..5): xt[64e + r64, (4c+sh)*T + t]
    # = x[t, 8*(64c+r64) + 4e + sh]
    xr = np.ascontiguousarray(x.T[:3072]).reshape(C16, 64, 2, 4, T)
    xt = np.ascontiguousarray(xr.transpose(2, 1, 0, 3, 4))    # [e,r64,c,sh,t]
    xt = xt.reshape(128, C16 * 4 * T).astype(np.float16)

    # e4m3 x pairs (all 16 (c, sh'') slots; adapters use slots 12..15):
    # xt8[p, ((slot*2+th)*256) + j*128 + m] = xq8[th*128+m, k(p, slot, j)]
    # with k = 8*(64c + r64) + 4e + 2sh'' + j, slot = 2c + sh''
    xq8 = x.astype(e4m3)
    xq8f = xq8.astype(np.float32)
    x8r = np.ascontiguousarray(xq8.T).reshape(NCH, 64, 2, 2, 2, T)
    x8r = x8r.transpose(0, 3, 2, 1, 4, 5)                 # [c,sh'',e,r64,j,t]
    x8r = np.ascontiguousarray(x8r).reshape(NBSLOT, 128, 2, T)
    xt8 = np.ascontiguousarray(
        x8r.reshape(NBSLOT, 128, 2, 2, 128)               # [slot,p,j,th,m]
           .transpose(0, 3, 1, 2, 4))                     # [slot,th,p,j,m]
    xt8 = xt8.reshape(NBSLOT * 2, 128, 256).transpose(1, 0, 2)
    xt8 = np.ascontiguousarray(xt8).reshape(128, NBSLOT * 2 * 256)

    # host-side zeros/centering correction (f32, added during unshard):
    # corr[d, t, n] = sum_g xs_cls[t, g] * s[d, g, n] * zeff[d, g, n]
    jj = 4 * np.arange(8, dtype=np.int32)
    z1 = ((qzeros[:, :, :, None] >> jj[None, None, None, :]) & 0xF)
    z1 = z1.reshape(D, G, OUT).astype(np.float32) + 1.0
    zeff = z1.copy()
    zeff[:, 24:, :] -= 8.0
    x16f = x.astype(np.float16).astype(np.float32)
    xs = np.empty((T, G), np.float32)
    xs[:, :24] = x16f[:, :3072].reshape(T, 24, GROUP).sum(axis=2)
    xs[:, 24:] = xq8f[:, 3072:].reshape(T, 8, GROUP).sum(axis=2)
    szf = scales * zeff                                       # [d, g, n]
    corr = np.einsum('tg,dgo->dto', xs, szf, optimize=True)   # [d, t, n] f32

    # DR-class e4m3 planes: k in [3072, 4096)
    w4hi = ((qweight[:, 384:512, None, :] >> jj[None, None, :, None]) & 0xF)
    w4hi = w4hi.reshape(D, 1024, OUT).astype(np.float32)      # [d, k', n]
    g_hi = 24 + np.arange(1024) // GROUP
    shi = scales[:, g_hi, :]
    p8full = (SCL * shi * (w4hi - 8.0)).astype(e4m3)          # [d, k', n]

    # base weights, e4m3 DoubleRow layout (64x):
    # wb8[p, slot*1024 + j*512 + n] = e4m3(64*bw.T[k(p, slot, j), n])
    bw8 = (SCL * base_weight.T).astype(e4m3)                  # [k, n]

    in_maps = []
    for cc in range(NCORES):
        ns = slice(cc * NC_OUT, (cc + 1) * NC_OUT)

        # fp16-class packed weights (chunks 0..5)
        qw_c = np.ascontiguousarray(qweight[:, :384, ns])     # [D, 384, 512]
        qw16 = qw_c.view(np.int16).reshape(D, C16, 64, NC_OUT, 2)
        qw16 = np.ascontiguousarray(qw16.transpose(0, 4, 2, 1, 3))
        qw16 = qw16.reshape(D, 128, FD16)
        qw16 = qw16.reshape(D, 128, 2, FD16 // 2).transpose(0, 2, 1, 3)

        # scale tile (64x): s2[d, p, c*512+n] = 64*s[d, 4c + (p%64)//16, n]
        s_c = SCL * scales[:, :24, ns]                        # [D, 24, 512]
        s2 = s_c.reshape(D, C16, 4, NC_OUT)
        s2 = np.repeat(s2, 16, axis=2)
        s2 = np.broadcast_to(s2[:, None], (D, 2, C16, 64, NC_OUT))
        s2 = np.ascontiguousarray(s2.transpose(0, 1, 3, 2, 4))
        s2 = s2.reshape(D, 128, FD16).astype(np.float16)
        s2 = s2.reshape(D, 128, 2, FD16 // 2).transpose(0, 2, 1, 3)

        # adapter DR planes: p8[d, p, slot*1024 + j*512 + n],
        # k = 8*(64*(6+rc) + r64) + 4e + 2sh'' + j, slot = 2rc + sh''
        p8c = p8full[:, :, ns]                                # [d, 1024, 512]
        p8r = p8c.reshape(D, 2, 64, 2, 2, 2, NC_OUT)       # [d,rc,r64,e,s,j,n]
        p8r = np.ascontiguousarray(p8r.transpose(0, 3, 2, 1, 4, 5, 6))
        # [d, e, r64, rc, sh'', j, n]
        p8v = p8r.reshape(D, 128, NSLOT * 2 * NC_OUT)

        # base DR planes: all 16 slots, split into 2 DMA halves
        b8c = bw8[:, ns]                                      # [4096, 512]
        b8r = b8c.reshape(NCH, 64, 2, 2, 2, NC_OUT)        # [c,r64,e,s,j,n]
        b8r = np.ascontiguousarray(b8r.transpose(2, 1, 0, 3, 4, 5))
        # [e, r64, c, sh'', j, n]
        wb8 = b8r.reshape(128, NBSLOT * 2 * NC_OUT)
        wb8 = np.ascontiguousarray(
            wb8.reshape(128, 2, 8 * 1024).transpose(1, 0, 2))

        in_maps.append({
            "xt": xt, "xt8": xt8,
            "qw16": np.ascontiguousarray(qw16),
            "s2": np.ascontiguousarray(s2),
            "p8": np.ascontiguousarray(p8v),
            "wb8": wb8,
        })
    return in_maps, corr


def _run(in_maps, trace=False):
    from concourse import bass_utils
    if "nc" not in _PROGRAM_CACHE:
        _PROGRAM_CACHE["nc"] = _build_program()
    nc = _PROGRAM_CACHE["nc"]
    res = bass_utils.run_bass_kernel_spmd(
        nc, in_maps, core_ids=list(range(NCORES)), trace=trace
    )
    return res


def kernel(x, base_weight, qweight, qzeros, scales, g_idx, _trace=False,
           _return_results=False):
    in_maps, corr = _prep_inputs(x, base_weight, qweight, qzeros, scales)
    res = _run(in_maps, trace=_trace)
    out = np.concatenate(
        [res.results[c]["out"].astype(np.float32)
         + res.results[c]["outb"].astype(np.float32)[None, :, :]
         for c in range(NCORES)], axis=2)
    out -= corr
    if _return_results:
        return out, res
    return out


# revision 14
# speedup vs baseline: 1.0025x; 1.0025x over previous
"""ColumnParallelLinearWithDelta: GPTQ-int4 LoRA-delta matmul on 8 trn2 cores.

out[d] = x @ dequant(qweight[d], qzeros[d], scales[d]) + x @ base_weight.T

Sharding: column-parallel — out_features (4096) split into 8 slices of 512,
one per NeuronCore; x replicated. Each core computes its [8, 256, 512] slice
of the delta stack plus the shared base output; the host adds base and the
K=32 zeros/centering correction during the unshard (exact f32, O(output)
plus a tiny [T,G]x[G,OUT] correction matmul per adapter).

Mixed-precision contraction (per core, out-col slice ns):
  W[k, n] = s[g(k), n] * (w4[k, n] - (z4[g(k), n] + 1)),  g(k) = k // 128

  k in [0, 3072): fp16 path:
    one tensor_scalar (>> 4sh & 0xF) per nibble plane over [128, 3072],
    tensor_tensor * (64*s) -> fp16 planes feed N=512 matmuls (rhs
    moving) against stationary fp16 x tiles; 48 matmuls/adapter.

  k in [3072, 4096): fp8 DoubleRow path:
    the host pre-dequantizes e4m3 planes p8 = e4m3(64*s*(w4-8)) — same
    HBM bytes as packed-int4 + expanded scales (1B/elem) but zero DVE
    work — and e4m3 x tiles paired two k-rows per PE cell
    (perf_mode=DoubleRow, lhsT [128,2,128], rhs [128,2,512]): each
    matmul contracts 256 k-rows in the cycles of 128, halving PE time
    for this range; 8 matmuls/adapter. The -8 nibble centering halves
    the e4m3 rounding error; its linear term and the GPTQ zeros fold
    into the host-side correction.

  The base matmul runs fully in DoubleRow e4m3 (wb8 = e4m3(64*bw), x
  pairs): 32 matmuls instead of 64. Everything accumulates at 64x scale
  in f32 PSUM; ScalarE drains apply 1/64 and write fp16 outputs.

Schedule: short N=128 PE warm-up (HAM un-throttle) -> adapter 0 (DR
matmuls first: no DVE dependency) -> base stream (pure-PE window that
lets DVE run an adapter ahead) -> adapters 1..7 with next-adapter weight
DMAs issued BEFORE the current adapter's matmul tail (keeps the sync
HWDGE ring's FIFO from serializing weights behind output completions).
Output DMAs ride the scalar ring; the final drain alternates rings and
adapter 7 runs t-half-major so th0's drain+DMA overlap th1's matmuls.
"""

import numpy as np
import ml_dtypes

# ---- problem constants (hardcoded; kernel.py must be self-contained) ----
T = 256          # tokens
IN = 4096        # in_features
OUT = 4096       # out_features
D = 8            # adapters
GROUP = 128      # quant group size
G = IN // GROUP  # 32 groups
NCORES = 8
NC_OUT = OUT // NCORES   # 512 out cols per core
NCH = 8                  # contraction chunks of 64 packed rows (x2 e-halves)
C16 = 6                  # chunks 0..5: fp16 path
FD16 = C16 * NC_OUT      # 3072 halfwords per partition (fp16 class)
NSLOT = 4                # adapter DR slots: (c in {6,7}) x (sh'' in {0,1})
NBSLOT = 16              # base DR slots: all (c, sh'')
SCL = 64.0               # psum scale (e4m3 range centering)

_PROGRAM_CACHE: dict = {}


def _build_program():
    import concourse.bacc as bacc
    import concourse.mybir as mybir
    import concourse.tile as tile

    nc = bacc.Bacc("TRN2", target_bir_lowering=False, debug=False)

    fp16 = mybir.dt.float16
    fp8 = mybir.dt.float8e4
    d_xt = nc.dram_tensor("xt", (128, C16 * 4 * T), fp16,
                          kind="ExternalInput")
    d_xt8 = nc.dram_tensor("xt8", (128, NBSLOT * 2 * 256), fp8,
                           kind="ExternalInput")
    d_qw16 = nc.dram_tensor(
        "qw16", (D, 2, 128, FD16 // 2), mybir.dt.int16, kind="ExternalInput"
    )
    d_s2 = nc.dram_tensor("s2", (D, 2, 128, FD16 // 2), fp16,
                          kind="ExternalInput")
    d_p8 = nc.dram_tensor("p8", (D, 128, NSLOT * 1024), fp8,
                          kind="ExternalInput")
    d_wb8 = nc.dram_tensor("wb8", (2, 128, 8 * 1024), fp8,
                           kind="ExternalInput")
    d_out = nc.dram_tensor("out", (D, T, NC_OUT), fp16, kind="ExternalOutput")
    d_outb = nc.dram_tensor("outb", (T, NC_OUT), fp16, kind="ExternalOutput")

    AT = mybir.AluOpType
    AF = mybir.ActivationFunctionType
    DR = mybir.MatmulPerfMode.DoubleRow

    with tile.TileContext(nc) as tc:
        with (
            tc.tile_pool(name="const", bufs=1) as cpool,
            tc.tile_pool(name="qw", bufs=2) as qpool,
            tc.tile_pool(name="s2", bufs=2) as spool,
            tc.tile_pool(name="p8", bufs=2) as p8pool,
            tc.tile_pool(name="vr", bufs=1) as vrpool,
            tc.tile_pool(name="v", bufs=2) as vpool,
            tc.tile_pool(name="wb", bufs=2) as wpool,
            tc.tile_pool(name="outp", bufs=4) as opool,
            tc.tile_pool(name="ps", bufs=2, space="PSUM") as ppool,
            tc.tile_pool(name="psb", bufs=1, space="PSUM") as pbpool,
        ):
            xt_sb = cpool.tile([128, C16 * 4 * T], fp16)
            xt8_sb = cpool.tile([128, NBSLOT * 2 * 256], fp8)
            warm_sb = cpool.tile([128, 640], fp16)

            def xt_tile(c, sh, th):
                off = (c * 4 + sh) * T + th * 128
                return xt_sb[:, off:off + 128]

            def xt8_pair(slot, th):
                off = (slot * 2 + th) * 256
                return xt8_sb[:, off:off + 256].rearrange(
                    "p (two m) -> p two m", two=2)

            def load_adapter(d):
                """DMA an adapter's weight streams. For adapter 0 the
                dequant pipeline is the latency-critical path (warm-up
                matmuls hide the DR planes), so qw/s2 go first; later
                adapters put the e4m3 planes first — they gate the
                adapter's first matmuls."""
                p8_t = p8pool.tile([128, NSLOT * 1024], fp8, name="p8_t")
                qw_t = qpool.tile([128, FD16], mybir.dt.int16, name="qw_t")
                s2_t = spool.tile([128, FD16], fp16, name="s2_t")
                hs = FD16 // 2
                if d > 0:
                    nc.sync.dma_start(p8_t[:], d_p8[d, :, :])
                for h in range(2):
                    nc.sync.dma_start(qw_t[:, h * hs:(h + 1) * hs],
                                      d_qw16[d, h, :, :])
                    nc.sync.dma_start(s2_t[:, h * hs:(h + 1) * hs],
                                      d_s2[d, h, :, :])
                    if d == 0 and h == 0:
                        nc.sync.dma_start(p8_t[:], d_p8[d, :, :])
                return p8_t, qw_t, s2_t

            def dr_mms(d, ps, p8_t, start):
                """8 DoubleRow matmuls (k in [3072, 4096)): no DVE dep."""
                for slot in range(NSLOT):
                    rhs = p8_t[:, slot * 1024:(slot + 1) * 1024].rearrange(
                        "p (two n) -> p two n", two=2)
                    for th in range(2):
                        nc.tensor.matmul(
                            ps[th][:], lhsT=xt8_pair(12 + slot, th), rhs=rhs,
                            start=(start and slot == 0), stop=False,
                            perf_mode=DR,
                        )

            def dequant16(d, qw_t, s2_t, n_parts=1):
                """fp16-class dequant: 4 TS + 4 TT over [128, 3072]."""
                vs = {}
                cs = FD16 // n_parts
                for part in range(n_parts):
                    c0 = part * cs
                    for sh in range(4):
                        vr = vrpool.tile([128, FD16], mybir.dt.int16,
                                         tag=f"vr{sh}", name=f"vr{sh}")
                        ts_i = nc.vector.tensor_scalar(
                            out=vr[:, c0:c0 + cs], in0=qw_t[:, c0:c0 + cs],
                            scalar1=4 * sh, scalar2=0xF,
                            op0=AT.logical_shift_right, op1=AT.bitwise_and,
                        )
                        v = vpool.tile([128, FD16], fp16, tag=f"v{sh}",
                                       name=f"v{sh}")
                        nc.vector.tensor_tensor(
                            out=v[:, c0:c0 + cs], in0=vr[:, c0:c0 + cs],
                            in1=s2_t[:, c0:c0 + cs], op=AT.mult
                        )
                        vs[(part, sh)] = v
                        if d == 0 and part == 0 and sh == 0:
                            first_ops[0] = ts_i
                return vs

            def fp16_mms(ps, vs, n_parts, ths, stop=False):
                ch_per = C16 // n_parts
                for part in range(n_parts):
                    for sh in range(4):
                        v = vs[(part, sh)]
                        for c in range(part * ch_per, (part + 1) * ch_per):
                            rhs = v[:, c * NC_OUT:(c + 1) * NC_OUT]
                            last = (part == n_parts - 1 and sh == 3
                                    and c == (part + 1) * ch_per - 1)
                            for th in ths:
                                nc.tensor.matmul(
                                    ps[th][:], lhsT=xt_tile(c, sh, th),
                                    rhs=rhs, start=False,
                                    stop=(stop and last),
                                )

            def drain_th(d, ps, th, ring_sync=False):
                """1/64-scaled ScalarE drain + fp16 output DMA."""
                o_t = opool.tile([128, NC_OUT], fp16, name="o_t")
                nc.scalar.activation(o_t[:], ps[th][:], AF.Identity,
                                     scale=1.0 / SCL)
                eng = nc.sync if ring_sync else nc.scalar
                eng.dma_start(d_out[d, th * 128:(th + 1) * 128, :], o_t[:])

            # ---- schedule ----
            first_ops = [None]
            loaded = load_adapter(0)
            nc.scalar.dma_start(xt8_sb[:], d_xt8[:])
            for q in range(4):
                nc.scalar.dma_start(
                    xt_sb[:, q * 6 * T:(q + 1) * 6 * T],
                    d_xt[:, q * 6 * T:(q + 1) * 6 * T])

            # PE warm-up: short matmuls flip the HAM clock gate to 2.4GHz
            # while the first weight transfers land
            nc.gpsimd.memset(warm_sb[:], 0.0)
            with tc.tile_pool(name="warmps", bufs=1, space="PSUM") as wpsp:
                warm_ps = wpsp.tile([128, 128], mybir.dt.float32)
                for _ in range(38):
                    nc.tensor.matmul(
                        warm_ps[:], lhsT=warm_sb[:, :128],
                        rhs=warm_sb[:, 128:256],
                        start=True, stop=True,
                    )

            ps0 = [ppool.tile([128, NC_OUT], mybir.dt.float32, tag=f"ps{t}",
                              name=f"ps{t}") for t in range(2)]
            p8_0, qw_0, s2_0 = loaded
            dr_mms(0, ps0, p8_0, start=True)
            vs0 = dequant16(0, qw_0, s2_0, n_parts=2)
            # base weights stream behind adapter 0's (sync-ring FIFO
            # already orders them after adapter 0's weights)
            wb_t = [wpool.tile([128, 8 * 1024], fp8, name=f"wb_t{h}",
                               tag=f"wb{h}") for h in range(2)]
            for h in range(2):
                nc.sync.dma_start(wb_t[h][:], d_wb8[h, :, :])
            loaded = load_adapter(1)
            fp16_mms(ps0, vs0, 2, (0, 1), stop=True)
            drain_th(0, ps0, 0)
            drain_th(0, ps0, 1)

            # base DoubleRow stream: DMA-only (no dequant), fills the PE
            # while DVE dequants adapter 1
            ps_b = [pbpool.tile([128, NC_OUT], mybir.dt.float32, tag=f"psb{t}",
                                name=f"psb{t}") for t in range(2)]
            for slot in range(NBSLOT):
                wt = wb_t[slot // 8]
                so = (slot % 8) * 1024
                rhs = wt[:, so:so + 1024].rearrange(
                    "p (two n) -> p two n", two=2)
                for th in range(2):
                    nc.tensor.matmul(
                        ps_b[th][:], lhsT=xt8_pair(slot, th), rhs=rhs,
                        start=(slot == 0), stop=(slot == NBSLOT - 1),
                        perf_mode=DR,
                    )
            for th in range(2):
                ob_t = opool.tile([128, NC_OUT], fp16, name="ob_t")
                nc.scalar.activation(ob_t[:], ps_b[th][:], AF.Identity,
                                     scale=1.0 / SCL)
                nc.scalar.dma_start(d_outb[th * 128:(th + 1) * 128, :],
                                    ob_t[:])

            for d in range(1, D):
                ps = [ppool.tile([128, NC_OUT], mybir.dt.float32, tag=f"ps{t}",
                                 name=f"ps{t}") for t in range(2)]
                p8_t, qw_t, s2_t = loaded
                dr_mms(d, ps, p8_t, start=True)
                vs = dequant16(d, qw_t, s2_t)
                if d < D - 1:
                    # prefetch next adapter's weights ahead of this
                    # adapter's matmul tail (sync-ring FIFO)
                    loaded = load_adapter(d + 1)
                    fp16_mms(ps, vs, 1, (0, 1), stop=True)
                    drain_th(d, ps, 0)
                    drain_th(d, ps, 1)
                else:
                    # last adapter: th-major so th0's drain + output DMA
                    # overlap th1's matmuls; the final drain is split in
                    # halves pipelined across both HWDGE rings to shrink
                    # the kernel tail
                    fp16_mms(ps, vs, 1, (0,), stop=True)
                    drain_th(d, ps, 0)
                    fp16_mms(ps, vs, 1, (1,), stop=True)
                    for half in range(2):
                        o_t = opool.tile([128, NC_OUT // 2], fp16,
                                         name="o_t")
                        cs = slice(half * 256, (half + 1) * 256)
                        nc.scalar.activation(o_t[:], ps[1][:, cs],
                                             AF.Identity, scale=1.0 / SCL)
                        eng = nc.scalar if half == 0 else nc.sync
                        eng.dma_start(d_out[d, 128:256, cs], o_t[:])

    nc.compile()
    return nc


def _prep_inputs(x, base_weight, qweight, qzeros, scales):
    """Host-side layout prep. Returns (per-core input maps, host corr)."""
    x = np.asarray(x, dtype=np.float32)
    base_weight = np.asarray(base_weight, dtype=np.float32)
    qweight = np.asarray(qweight, dtype=np.int32)
    qzeros = np.asarray(qzeros, dtype=np.int32)
    scales = np.asarray(scales, dtype=np.float32)
    e4m3 = ml_dtypes.float8_e4m3

    # stationary fp16 x tiles (chunks 0..5): xt[64e + r64, (4c+sh)*T + t]
    # = x[t, 8*(64c+r64) + 4e + sh]
    xr = np.ascontiguousarray(x.T[:3072]).reshape(C16, 64, 2, 4, T)
    xt = np.ascontiguousarray(xr.transpose(2, 1, 0, 3, 4))    # [e,r64,c,sh,t]
    xt = xt.reshape(128, C16 * 4 * T).astype(np.float16)

    # e4m3 x pairs (all 16 (c, sh'') slots; adapters use slots 12..15):
    # xt8[p, ((slot*2+th)*256) + j*128 + m] = xq8[th*128+m, k(p, slot, j)]
    # with k = 8*(64c + r64) + 4e + 2sh'' + j, slot = 2c + sh''
    xq8 = x.astype(e4m3)
    xq8f = xq8.astype(np.float32)
    x8r = np.ascontiguousarray(xq8.T).reshape(NCH, 64, 2, 2, 2, T)
    x8r = x8r.transpose(0, 3, 2, 1, 4, 5)                 # [c,sh'',e,r64,j,t]
    x8r = np.ascontiguousarray(x8r).reshape(NBSLOT, 128, 2, T)
    xt8 = np.ascontiguousarray(
        x8r.reshape(NBSLOT, 128, 2, 2, 128)               # [slot,p,j,th,m]
           .transpose(0, 3, 1, 2, 4))                     # [slot,th,p,j,m]
    xt8 = xt8.reshape(NBSLOT * 2, 128, 256).transpose(1, 0, 2)
    xt8 = np.ascontiguousarray(xt8).reshape(128, NBSLOT * 2 * 256)

    # host-side zeros/centering correction (f32, added during unshard):
    # corr[d, t, n] = sum_g xs_cls[t, g] * s[d, g, n] * zeff[d, g, n]
    jj = 4 * np.arange(8, dtype=np.int32)
    z1 = ((qzeros[:, :, :, None] >> jj[None, None, None, :]) & 0xF)
    z1 = z1.reshape(D, G, OUT).astype(np.float32) + 1.0
    zeff = z1.copy()
    zeff[:, 24:, :] -= 8.0
    x16f = x.astype(np.float16).astype(np.float32)
    xs = np.empty((T, G), np.float32)
    xs[:, :24] = x16f[:, :3072].reshape(T, 24, GROUP).sum(axis=2)
    xs[:, 24:] = xq8f[:, 3072:].reshape(T, 8, GROUP).sum(axis=2)
    szf = scales * zeff                                       # [d, g, n]
    corr = np.einsum('tg,dgo->dto', xs, szf, optimize=True)   # [d, t, n] f32

    # DR-class e4m3 planes: k in [3072, 4096)
    w4hi = ((qweight[:, 384:512, None, :] >> jj[None, None, :, None]) & 0xF)
    w4hi = w4hi.reshape(D, 1024, OUT).astype(np.float32)      # [d, k', n]
    g_hi = 24 + np.arange(1024) // GROUP
    shi = scales[:, g_hi, :]
    p8full = (SCL * shi * (w4hi - 8.0)).astype(e4m3)          # [d, k', n]

    # base weights, e4m3 DoubleRow layout (64x):
    # wb8[p, slot*1024 + j*512 + n] = e4m3(64*bw.T[k(p, slot, j), n])
    bw8 = (SCL * base_weight.T).astype(e4m3)                  # [k, n]

    in_maps = []
    for cc in range(NCORES):
        ns = slice(cc * NC_OUT, (cc + 1) * NC_OUT)

        # fp16-class packed weights (chunks 0..5)
        qw_c = np.ascontiguousarray(qweight[:, :384, ns])     # [D, 384, 512]
        qw16 = qw_c.view(np.int16).reshape(D, C16, 64, NC_OUT, 2)
        qw16 = np.ascontiguousarray(qw16.transpose(0, 4, 2, 1, 3))
        qw16 = qw16.reshape(D, 128, FD16)
        qw16 = qw16.reshape(D, 128, 2, FD16 // 2).transpose(0, 2, 1, 3)

        # scale tile (64x): s2[d, p, c*512+n] = 64*s[d, 4c + (p%64)//16, n]
        s_c = SCL * scales[:, :24, ns]                        # [D, 24, 512]
        s2 = s_c.reshape(D, C16, 4, NC_OUT)
        s2 = np.repeat(s2, 16, axis=2)
        s2 = np.broadcast_to(s2[:, None], (D, 2, C16, 64, NC_OUT))
        s2 = np.ascontiguousarray(s2.transpose(0, 1, 3, 2, 4))
        s2 = s2.reshape(D, 128, FD16).astype(np.float16)
        s2 = s2.reshape(D, 128, 2, FD16 // 2).transpose(0, 2, 1, 3)

        # adapter DR planes: p8[d, p, slot*1024 + j*512 + n],
        # k = 8*(64*(6+rc) + r64) + 4e + 2sh'' + j, slot = 2rc + sh''
        p8c = p8full[:, :, ns]                                # [d, 1024, 512]
        p8r = p8c.reshape(D, 2, 64, 2, 2, 2, NC_OUT)       # [d,rc,r64,e,s,j,n]
        p8r = np.ascontiguousarray(p8r.transpose(0, 3, 2, 1, 4, 5, 6))
        # [d, e, r64, rc, sh'', j, n]
        p8v = p8r.reshape(D, 128, NSLOT * 2 * NC_OUT)

        # base DR planes: all 16 slots, split into 2 DMA halves
        b8c = bw8[:, ns]                                      # [4096, 512]
        b8r = b8c.reshape(NCH, 64, 2, 2, 2, NC_OUT)        # [c,r64,e,s,j,n]
        b8r = np.ascontiguousarray(b8r.transpose(2, 1, 0, 3, 4, 5))
        # [e, r64, c, sh'', j, n]
        wb8 = b8r.reshape(128, NBSLOT * 2 * NC_OUT)
        wb8 = np.ascontiguousarray(
            wb8.reshape(128, 2, 8 * 1024).transpose(1, 0, 2))

        in_maps.append({
            "xt": xt, "xt8": xt8,
            "qw16": np.ascontiguousarray(qw16),
            "s2": np.ascontiguousarray(s2),
            "p8": np.ascontiguousarray(p8v),
            "wb8": wb8,
        })
    return in_maps, corr


def _run(in_maps, trace=False):
    from concourse import bass_utils
    if "nc" not in _PROGRAM_CACHE:
        _PROGRAM_CACHE["nc"] = _build_program()
    nc = _PROGRAM_CACHE["nc"]
    res = bass_utils.run_bass_kernel_spmd(
        nc, in_maps, core_ids=list(range(NCORES)), trace=trace
    )
    return res


def kernel(x, base_weight, qweight, qzeros, scales, g_idx, _trace=False,
           _return_results=False):
    in_maps, corr = _prep_inputs(x, base_weight, qweight, qzeros, scales)
    res = _run(in_maps, trace=_trace)
    out = np.concatenate(
        [res.results[c]["out"].astype(np.float32)
         + res.results[c]["outb"].astype(np.float32)[None, :, :]
         for c in range(NCORES)], axis=2)
    out -= corr
    if _return_results:
        return out, res
    return out


# revision 16
# speedup vs baseline: 1.0103x; 1.0077x over previous
"""ColumnParallelLinearWithDelta: GPTQ-int4 LoRA-delta matmul on 8 trn2 cores.

out[d] = x @ dequant(qweight[d], qzeros[d], scales[d]) + x @ base_weight.T

Sharding: column-parallel — out_features (4096) split into 8 slices of 512,
one per NeuronCore; x replicated. Each core computes its [8, 256, 512] slice
of the delta stack plus the shared base output; the host adds base and the
K=32 zeros/centering correction during the unshard (exact f32, O(output)
plus a tiny [T,G]x[G,OUT] correction matmul per adapter).

Mixed-precision contraction (per core, out-col slice ns):
  W[k, n] = s[g(k), n] * (w4[k, n] - (z4[g(k), n] + 1)),  g(k) = k // 128

  k in [0, 3072): fp16 path:
    one tensor_scalar (>> 4sh & 0xF) per nibble plane over [128, 3072],
    tensor_tensor * (64*s) -> fp16 planes feed N=512 matmuls (rhs
    moving) against stationary fp16 x tiles; 48 matmuls/adapter.

  k in [3072, 4096): fp8 DoubleRow path:
    the host pre-dequantizes e4m3 planes p8 = e4m3(64*s*(w4-8)) — same
    HBM bytes as packed-int4 + expanded scales (1B/elem) but zero DVE
    work — and e4m3 x tiles paired two k-rows per PE cell
    (perf_mode=DoubleRow, lhsT [128,2,128], rhs [128,2,512]): each
    matmul contracts 256 k-rows in the cycles of 128, halving PE time
    for this range; 8 matmuls/adapter. The -8 nibble centering halves
    the e4m3 rounding error; its linear term and the GPTQ zeros fold
    into the host-side correction.

  The base matmul runs fully in DoubleRow e4m3 (wb8 = e4m3(64*bw), x
  pairs): 32 matmuls instead of 64. Everything accumulates at 64x scale
  in f32 PSUM; ScalarE drains apply 1/64 and write fp16 outputs.

Schedule: short N=128 PE warm-up (HAM un-throttle) -> adapter 0 (DR
matmuls first: no DVE dependency) -> base stream (pure-PE window that
lets DVE run an adapter ahead) -> adapters 1..7 with next-adapter weight
DMAs issued BEFORE the current adapter's matmul tail (keeps the sync
HWDGE ring's FIFO from serializing weights behind output completions).
Output DMAs ride the scalar ring; the final drain alternates rings and
adapter 7 runs t-half-major so th0's drain+DMA overlap th1's matmuls.
"""

import numpy as np
import ml_dtypes

# ---- problem constants (hardcoded; kernel.py must be self-contained) ----
T = 256          # tokens
IN = 4096        # in_features
OUT = 4096       # out_features
D = 8            # adapters
GROUP = 128      # quant group size
G = IN // GROUP  # 32 groups
NCORES = 8
NC_OUT = OUT // NCORES   # 512 out cols per core
NCH = 8                  # contraction chunks of 64 packed rows (x2 e-halves)
C16 = 6                  # chunks 0..5: fp16 path
FD16 = C16 * NC_OUT      # 3072 halfwords per partition (fp16 class)
NSLOT = 4                # adapter DR slots: (c in {6,7}) x (sh'' in {0,1})
NBSLOT = 16              # base DR slots: all (c, sh'')
SCL = 64.0               # psum scale (e4m3 range centering)

_PROGRAM_CACHE: dict = {}


def _build_program():
    import concourse.bacc as bacc
    import concourse.mybir as mybir
    import concourse.tile as tile

    nc = bacc.Bacc("TRN2", target_bir_lowering=False, debug=False)

    fp16 = mybir.dt.float16
    fp8 = mybir.dt.float8e4
    d_xt = nc.dram_tensor("xt", (128, C16 * 4 * T), fp16,
                          kind="ExternalInput")
    d_xt8 = nc.dram_tensor("xt8", (128, NBSLOT * 2 * 256), fp8,
                           kind="ExternalInput")
    d_qw16 = nc.dram_tensor(
        "qw16", (D, 2, 128, FD16 // 2), mybir.dt.int16, kind="ExternalInput"
    )
    d_s2 = nc.dram_tensor("s2", (D, 2, 128, FD16 // 2), fp16,
                          kind="ExternalInput")
    d_p8 = nc.dram_tensor("p8", (D, 128, NSLOT * 1024), fp8,
                          kind="ExternalInput")
    d_wb8 = nc.dram_tensor("wb8", (2, 128, 8 * 1024), fp8,
                           kind="ExternalInput")
    d_out = nc.dram_tensor("out", (D, T, NC_OUT), fp16, kind="ExternalOutput")
    d_outb = nc.dram_tensor("outb", (T, NC_OUT), fp16, kind="ExternalOutput")

    AT = mybir.AluOpType
    AF = mybir.ActivationFunctionType
    DR = mybir.MatmulPerfMode.DoubleRow

    with tile.TileContext(nc) as tc:
        with (
            tc.tile_pool(name="const", bufs=1) as cpool,
            tc.tile_pool(name="qw", bufs=2) as qpool,
            tc.tile_pool(name="s2", bufs=2) as spool,
            tc.tile_pool(name="p8", bufs=2) as p8pool,
            tc.tile_pool(name="vr", bufs=1) as vrpool,
            tc.tile_pool(name="v", bufs=2) as vpool,
            tc.tile_pool(name="wb", bufs=2) as wpool,
            tc.tile_pool(name="outp", bufs=4) as opool,
            tc.tile_pool(name="ps", bufs=2, space="PSUM") as ppool,
            tc.tile_pool(name="psb", bufs=1, space="PSUM") as pbpool,
        ):
            xt_sb = cpool.tile([128, C16 * 4 * T], fp16)
            xt8_sb = cpool.tile([128, NBSLOT * 2 * 256], fp8)
            warm_sb = cpool.tile([128, 640], fp16)

            def xt_tile(c, sh, th):
                off = (c * 4 + sh) * T + th * 128
                return xt_sb[:, off:off + 128]

            def xt8_pair(slot, th):
                off = (slot * 2 + th) * 256
                return xt8_sb[:, off:off + 256].rearrange(
                    "p (two m) -> p two m", two=2)

            def load_adapter(d):
                """DMA an adapter's weight streams. For adapter 0 the
                dequant pipeline is the latency-critical path (warm-up
                matmuls hide the DR planes), so qw/s2 go first; later
                adapters put the e4m3 planes first — they gate the
                adapter's first matmuls."""
                p8_t = p8pool.tile([128, NSLOT * 1024], fp8, name="p8_t")
                qw_t = qpool.tile([128, FD16], mybir.dt.int16, name="qw_t")
                s2_t = spool.tile([128, FD16], fp16, name="s2_t")
                hs = FD16 // 2
                nc.sync.dma_start(p8_t[:], d_p8[d, :, :])
                for h in range(2):
                    nc.sync.dma_start(qw_t[:, h * hs:(h + 1) * hs],
                                      d_qw16[d, h, :, :])
                    nc.sync.dma_start(s2_t[:, h * hs:(h + 1) * hs],
                                      d_s2[d, h, :, :])
                return p8_t, qw_t, s2_t

            def dr_mms(d, ps, p8_t, start):
                """8 DoubleRow matmuls (k in [3072, 4096)): no DVE dep."""
                for slot in range(NSLOT):
                    rhs = p8_t[:, slot * 1024:(slot + 1) * 1024].rearrange(
                        "p (two n) -> p two n", two=2)
                    for th in range(2):
                        nc.tensor.matmul(
                            ps[th][:], lhsT=xt8_pair(12 + slot, th), rhs=rhs,
                            start=(start and slot == 0), stop=False,
                            perf_mode=DR,
                        )

            def dequant16(d, qw_t, s2_t, n_parts=1):
                """fp16-class dequant: 4 TS + 4 TT over [128, 3072]."""
                vs = {}
                cs = FD16 // n_parts
                for part in range(n_parts):
                    c0 = part * cs
                    for sh in range(4):
                        vr = vrpool.tile([128, FD16], mybir.dt.int16,
                                         tag=f"vr{sh}", name=f"vr{sh}")
                        ts_i = nc.vector.tensor_scalar(
                            out=vr[:, c0:c0 + cs], in0=qw_t[:, c0:c0 + cs],
                            scalar1=4 * sh, scalar2=0xF,
                            op0=AT.logical_shift_right, op1=AT.bitwise_and,
                        )
                        v = vpool.tile([128, FD16], fp16, tag=f"v{sh}",
                                       name=f"v{sh}")
                        nc.vector.tensor_tensor(
                            out=v[:, c0:c0 + cs], in0=vr[:, c0:c0 + cs],
                            in1=s2_t[:, c0:c0 + cs], op=AT.mult
                        )
                        vs[(part, sh)] = v
                        if d == 0 and part == 0 and sh == 0:
                            first_ops[0] = ts_i
                return vs

            def fp16_mms(ps, vs, n_parts, ths, stop=False):
                ch_per = C16 // n_parts
                for part in range(n_parts):
                    for sh in range(4):
                        v = vs[(part, sh)]
                        for c in range(part * ch_per, (part + 1) * ch_per):
                            rhs = v[:, c * NC_OUT:(c + 1) * NC_OUT]
                            last = (part == n_parts - 1 and sh == 3
                                    and c == (part + 1) * ch_per - 1)
                            for th in ths:
                                nc.tensor.matmul(
                                    ps[th][:], lhsT=xt_tile(c, sh, th),
                                    rhs=rhs, start=False,
                                    stop=(stop and last),
                                )

            def drain_th(d, ps, th, ring_sync=False):
                """1/64-scaled ScalarE drain + fp16 output DMA."""
                o_t = opool.tile([128, NC_OUT], fp16, name="o_t")
                nc.scalar.activation(o_t[:], ps[th][:], AF.Identity,
                                     scale=1.0 / SCL)
                eng = nc.sync if ring_sync else nc.scalar
                eng.dma_start(d_out[d, th * 128:(th + 1) * 128, :], o_t[:])

            # ---- schedule ----
            first_ops = [None]
            loaded = load_adapter(0)
            nc.scalar.dma_start(xt8_sb[:], d_xt8[:])
            for q in range(4):
                nc.scalar.dma_start(
                    xt_sb[:, q * 6 * T:(q + 1) * 6 * T],
                    d_xt[:, q * 6 * T:(q + 1) * 6 * T])

            # PE warm-up: short matmuls flip the HAM clock gate to 2.4GHz
            # while the first weight transfers land
            nc.gpsimd.memset(warm_sb[:], 0.0)
            with tc.tile_pool(name="warmps", bufs=1, space="PSUM") as wpsp:
                warm_ps = wpsp.tile([128, 128], mybir.dt.float32)
                for _ in range(34):
                    nc.tensor.matmul(
                        warm_ps[:], lhsT=warm_sb[:, :128],
                        rhs=warm_sb[:, 128:256],
                        start=True, stop=True,
                    )

            ps0 = [ppool.tile([128, NC_OUT], mybir.dt.float32, tag=f"ps{t}",
                              name=f"ps{t}") for t in range(2)]
            p8_0, qw_0, s2_0 = loaded
            dr_mms(0, ps0, p8_0, start=True)
            vs0 = dequant16(0, qw_0, s2_0, n_parts=2)
            # base weights stream behind adapter 0's (sync-ring FIFO
            # already orders them after adapter 0's weights)
            wb_t = [wpool.tile([128, 8 * 1024], fp8, name=f"wb_t{h}",
                               tag=f"wb{h}") for h in range(2)]
            for h in range(2):
                nc.sync.dma_start(wb_t[h][:], d_wb8[h, :, :])
            loaded = load_adapter(1)
            fp16_mms(ps0, vs0, 2, (0, 1), stop=True)
            drain_th(0, ps0, 0)
            drain_th(0, ps0, 1)

            # base DoubleRow stream: DMA-only (no dequant), fills the PE
            # while DVE dequants adapter 1
            ps_b = [pbpool.tile([128, NC_OUT], mybir.dt.float32, tag=f"psb{t}",
                                name=f"psb{t}") for t in range(2)]
            for slot in range(NBSLOT):
                wt = wb_t[slot // 8]
                so = (slot % 8) * 1024
                rhs = wt[:, so:so + 1024].rearrange(
                    "p (two n) -> p two n", two=2)
                for th in range(2):
                    nc.tensor.matmul(
                        ps_b[th][:], lhsT=xt8_pair(slot, th), rhs=rhs,
                        start=(slot == 0), stop=(slot == NBSLOT - 1),
                        perf_mode=DR,
                    )
            for th in range(2):
                ob_t = opool.tile([128, NC_OUT], fp16, name="ob_t")
                nc.scalar.activation(ob_t[:], ps_b[th][:], AF.Identity,
                                     scale=1.0 / SCL)
                nc.scalar.dma_start(d_outb[th * 128:(th + 1) * 128, :],
                                    ob_t[:])

            for d in range(1, D):
                ps = [ppool.tile([128, NC_OUT], mybir.dt.float32, tag=f"ps{t}",
                                 name=f"ps{t}") for t in range(2)]
                p8_t, qw_t, s2_t = loaded
                dr_mms(d, ps, p8_t, start=True)
                vs = dequant16(d, qw_t, s2_t)
                if d < D - 1:
                    # prefetch next adapter's weights ahead of this
                    # adapter's matmul tail (sync-ring FIFO)
                    loaded = load_adapter(d + 1)
                    fp16_mms(ps, vs, 1, (0, 1), stop=True)
                    drain_th(d, ps, 0)
                    drain_th(d, ps, 1)
                else:
                    # last adapter: th-major so th0's drain + output DMA
                    # overlap th1's matmuls; the final drain is split in
                    # halves pipelined across both HWDGE rings to shrink
                    # the kernel tail
                    fp16_mms(ps, vs, 1, (0,), stop=True)
                    drain_th(d, ps, 0)
                    fp16_mms(ps, vs, 1, (1,), stop=True)
                    for half in range(2):
                        o_t = opool.tile([128, NC_OUT // 2], fp16,
                                         name="o_t")
                        cs = slice(half * 256, (half + 1) * 256)
                        nc.scalar.activation(o_t[:], ps[1][:, cs],
                                             AF.Identity, scale=1.0 / SCL)
                        eng = nc.scalar if half == 0 else nc.sync
                        eng.dma_start(d_out[d, 128:256, cs], o_t[:])

    nc.compile()
    return nc


def _prep_inputs(x, base_weight, qweight, qzeros, scales):
    """Host-side layout prep. Returns (per-core input maps, host corr)."""
    x = np.asarray(x, dtype=np.float32)
    base_weight = np.asarray(base_weight, dtype=np.float32)
    qweight = np.asarray(qweight, dtype=np.int32)
    qzeros = np.asarray(qzeros, dtype=np.int32)
    scales = np.asarray(scales, dtype=np.float32)
    e4m3 = ml_dtypes.float8_e4m3

    # stationary fp16 x tiles (chunks 0..5): xt[64e + r64, (4c+sh)*T + t]
    # = x[t, 8*(64c+r64) + 4e + sh]
    xr = np.ascontiguousarray(x.T[:3072]).reshape(C16, 64, 2, 4, T)
    xt = np.ascontiguousarray(xr.transpose(2, 1, 0, 3, 4))    # [e,r64,c,sh,t]
    xt = xt.reshape(128, C16 * 4 * T).astype(np.float16)

    # e4m3 x pairs (all 16 (c, sh'') slots; adapters use slots 12..15):
    # xt8[p, ((slot*2+th)*256) + j*128 + m] = xq8[th*128+m, k(p, slot, j)]
    # with k = 8*(64c + r64) + 4e + 2sh'' + j, slot = 2c + sh''
    xq8 = x.astype(e4m3)
    xq8f = xq8.astype(np.float32)
    x8r = np.ascontiguousarray(xq8.T).reshape(NCH, 64, 2, 2, 2, T)
    x8r = x8r.transpose(0, 3, 2, 1, 4, 5)                 # [c,sh'',e,r64,j,t]
    x8r = np.ascontiguousarray(x8r).reshape(NBSLOT, 128, 2, T)
    xt8 = np.ascontiguousarray(
        x8r.reshape(NBSLOT, 128, 2, 2, 128)               # [slot,p,j,th,m]
           .transpose(0, 3, 1, 2, 4))                     # [slot,th,p,j,m]
    xt8 = xt8.reshape(NBSLOT * 2, 128, 256).transpose(1, 0, 2)
    xt8 = np.ascontiguousarray(xt8).reshape(128, NBSLOT * 2 * 256)

    # host-side zeros/centering correction (f32, added during unshard):
    # corr[d, t, n] = sum_g xs_cls[t, g] * s[d, g, n] * zeff[d, g, n]
    jj = 4 * np.arange(8, dtype=np.int32)
    z1 = ((qzeros[:, :, :, None] >> jj[None, None, None, :]) & 0xF)
    z1 = z1.reshape(D, G, OUT).astype(np.float32) + 1.0
    zeff = z1.copy()
    zeff[:, 24:, :] -= 8.0
    x16f = x.astype(np.float16).astype(np.float32)
    xs = np.empty((T, G), np.float32)
    xs[:, :24] = x16f[:, :3072].reshape(T, 24, GROUP).sum(axis=2)
    xs[:, 24:] = xq8f[:, 3072:].reshape(T, 8, GROUP).sum(axis=2)
    szf = scales * zeff                                       # [d, g, n]
    corr = np.einsum('tg,dgo->dto', xs, szf, optimize=True)   # [d, t, n] f32

    # DR-class e4m3 planes: k in [3072, 4096)
    w4hi = ((qweight[:, 384:512, None, :] >> jj[None, None, :, None]) & 0xF)
    w4hi = w4hi.reshape(D, 1024, OUT).astype(np.float32)      # [d, k', n]
    g_hi = 24 + np.arange(1024) // GROUP
    shi = scales[:, g_hi, :]
    p8full = (SCL * shi * (w4hi - 8.0)).astype(e4m3)          # [d, k', n]

    # base weights, e4m3 DoubleRow layout (64x):
    # wb8[p, slot*1024 + j*512 + n] = e4m3(64*bw.T[k(p, slot, j), n])
    bw8 = (SCL * base_weight.T).astype(e4m3)                  # [k, n]

    in_maps = []
    for cc in range(NCORES):
        ns = slice(cc * NC_OUT, (cc + 1) * NC_OUT)

        # fp16-class packed weights (chunks 0..5)
        qw_c = np.ascontiguousarray(qweight[:, :384, ns])     # [D, 384, 512]
        qw16 = qw_c.view(np.int16).reshape(D, C16, 64, NC_OUT, 2)
        qw16 = np.ascontiguousarray(qw16.transpose(0, 4, 2, 1, 3))
        qw16 = qw16.reshape(D, 128, FD16)
        qw16 = qw16.reshape(D, 128, 2, FD16 // 2).transpose(0, 2, 1, 3)

        # scale tile (64x): s2[d, p, c*512+n] = 64*s[d, 4c + (p%64)//16, n]
        s_c = SCL * scales[:, :24, ns]                        # [D, 24, 512]
        s2 = s_c.reshape(D, C16, 4, NC_OUT)
        s2 = np.repeat(s2, 16, axis=2)
        s2 = np.broadcast_to(s2[:, None], (D, 2, C16, 64, NC_OUT))
        s2 = np.ascontiguousarray(s2.transpose(0, 1, 3, 2, 4))
        s2 = s2.reshape(D, 128, FD16).astype(np.float16)
        s2 = s2.reshape(D, 128, 2, FD16 // 2).transpose(0, 2, 1, 3)

        # adapter DR planes: p8[d, p, slot*1024 + j*512 + n],
        # k = 8*(64*(6+rc) + r64) + 4e + 2sh'' + j, slot = 2rc + sh''
        p8c = p8full[:, :, ns]                                # [d, 1024, 512]
        p8r = p8c.reshape(D, 2, 64, 2, 2, 2, NC_OUT)       # [d,rc,r64,e,s,j,n]
        p8r = np.ascontiguousarray(p8r.transpose(0, 3, 2, 1, 4, 5, 6))
        # [d, e, r64, rc, sh'', j, n]
        p8v = p8r.reshape(D, 128, NSLOT * 2 * NC_OUT)

        # base DR planes: all 16 slots, split into 2 DMA halves
        b8c = bw8[:, ns]                                      # [4096, 512]
        b8r = b8c.reshape(NCH, 64, 2, 2, 2, NC_OUT)        # [c,r64,e,s,j,n]
        b8r = np.ascontiguousarray(b8r.transpose(2, 1, 0, 3, 4, 5))
        # [e, r64, c, sh'', j, n]
        wb8 = b8r.reshape(128, NBSLOT * 2 * NC_OUT)
        wb8 = np.ascontiguousarray(
            wb8.reshape(128, 2, 8 * 1024).transpose(1, 0, 2))

        in_maps.append({
            "xt": xt, "xt8": xt8,
            "qw16": np.ascontiguousarray(qw16),
            "s2": np.ascontiguousarray(s2),
            "p8": np.ascontiguousarray(p8v),
            "wb8": wb8,
        })
    return in_maps, corr


def _run(in_maps, trace=False):
    from concourse import bass_utils
    if "nc" not in _PROGRAM_CACHE:
        _PROGRAM_CACHE["nc"] = _build_program()
    nc = _PROGRAM_CACHE["nc"]
    res = bass_utils.run_bass_kernel_spmd(
        nc, in_maps, core_ids=list(range(NCORES)), trace=trace
    )
    return res


def kernel(x, base_weight, qweight, qzeros, scales, g_idx, _trace=False,
           _return_results=False):
    in_maps, corr = _prep_inputs(x, base_weight, qweight, qzeros, scales)
    res = _run(in_maps, trace=_trace)
    out = np.concatenate(
        [res.results[c]["out"].astype(np.float32)
         + res.results[c]["outb"].astype(np.float32)[None, :, :]
         for c in range(NCORES)], axis=2)
    out -= corr
    if _return_results:
        return out, res
    return out
